# revision 56
# baseline (speedup 1.0000x reference)
"""Trainium2 Bass kernel for nn_AUV_39565238730960.

Computation (per coil c, sharded 1 coil per NeuronCore over 8 cores):
    Z_b   = x_b * csm_c                 (complex elementwise, 30 images)
    Y_b   = T @ Z_b @ T                 (centered ortho 2D FFT as matmuls,
                                         T = symmetric centered DFT matrix)
    Yr    = interleave(Re Y, Im Y)      (30, 131072)
    out_c = mask * (VT^T @ Yr)          (200, 131072) -> stored fp16

v9 design notes (timeline-cost-model driven; ~355us vs 420us baseline):
  - Two phases. A: the full FFT for all 30 images (pass L + both pass-R
    halves), software-pipelined (L(b+1) emitted before R(b)) so the PE
    runs its 16 matmuls/image back-to-back at ~100% with the wt0-copy
    latency hidden; DVE runs the complex coil multiply a few images
    ahead (deep per-temp tile pools break WAR-semaphore lockstep).
    B: pure projection over both k halves -- no FFT/projection engine or
    queue contention anywhere.
  - The sim serializes all DMA on one resource at ~360 GB/s: out 145.6us
    + mask 72.8 + x 21.8 + corner 21.8 = 264.5us. Phase B is DMA-bound,
    so phase A absorbs every byte it can: x loads, all corner turns, and
    3 prefetched mask pair-tiles (SBUF-capped); later pairs are issued
    mid-strip with ~a strip of lead time.
  - DMA issue itself is costly (~1.2-1.4us of issuing-queue hold plus a
    serial ~630ns HWDGE stage per DMA): mask loads ride SP as 8192-col
    pairs, out stores ride SP, corner turns ride Act (program-ordered
    right after their yb copy so the queue head never stalls), x loads
    ride Pool/SWDGE alone (gen cost lands in Pool's idle phase A).
  - Elementwise cost = free-size x cycle_t (partition count is free).
    The masked PSUM eviction (f32 psum x u8 mask -> f16) can only run on
    DVE directly (Act has no tensor_tensor; Pool cannot read PSUM).
    Projection psums are 2-bank [128|72, 1024] units routed 28/37/35
    across {DVE-direct, Act-copy+DVE-mult, Act-copy+Pool-mult}, which
    equalizes DVE/Act/Pool drain at ~192us -- just under phase B's DMA.
  - Per-image pass-R output is corner-turned with one SBUF->SBUF DMA per
    (image, kt) into resident strip tiles yr_kt[32*s + b, r*512 + c], so
    the projection's moving operand sits at 32-aligned partition bases.
"""

import numpy as np

NCH, NBASIS, NXS, NF = 8, 30, 256, 200
NX = NXS * NXS * 2

_CACHE = {}


def _fmat():
    """Symmetric centered orthonormal DFT matrix: fft1c(z) = T @ z."""
    eye = np.eye(NXS, dtype=np.complex128)
    t = np.fft.fftshift(
        np.fft.fft(np.fft.ifftshift(eye, axes=0), axis=0, norm="ortho"), axes=0
    )
    return t


def _build():
    import concourse.bacc as bacc
    import concourse.mybir as mybir
    import concourse.tile as tile

    F32 = mybir.dt.float32
    F16 = mybir.dt.float16
    U8 = mybir.dt.uint8
    MULT = mybir.AluOpType.mult
    ADD = mybir.AluOpType.add
    SUB = mybir.AluOpType.subtract

    t = _fmat()
    tr = t.real.astype(np.float32).reshape(2, 128, NXS).transpose(1, 0, 2)
    ti = t.imag.astype(np.float32).reshape(2, 128, NXS).transpose(1, 0, 2)
    # stacked moving operands (128, 2, 512): [Tr | Ti] and [-Ti | Tr]
    f_a = np.concatenate([tr, ti], axis=2).astype(np.float16)
    f_b = np.concatenate([-ti, tr], axis=2).astype(np.float16)

    nc = bacc.Bacc("TRN2", target_bir_lowering=False, debug=False, num_devices=NCH)

    # x / csm arrive with re/im planes separated: [.., 128, 2(rh), 2(reim), 256]
    x_d = nc.dram_tensor("x", [NBASIS, 128, 2, 2, NXS], F16, kind="ExternalInput")
    c_d = nc.dram_tensor("csm", [128, 2, 2, NXS], F16, kind="ExternalInput")
    v_d = nc.dram_tensor("vt", [NBASIS, NF], F32, kind="ExternalInput")
    m_d = nc.dram_tensor("mask", [NF, NX], U8, kind="ExternalInput")
    o_d = nc.dram_tensor("out", [NF, NX], F16, kind="ExternalOutput")

    fa_d = nc.inline_tensor(f_a, "fmat_a")
    fb_d = nc.inline_tensor(f_b, "fmat_b")

    MCHUNK = 4096  # mask / out tile width

    with tile.TileContext(nc) as tc:
        with (
            tc.tile_pool(name="const", bufs=1) as cpool,
            tc.tile_pool(name="work", bufs=1) as wpool,
            tc.tile_pool(name="psum", bufs=1, space="PSUM") as psum,
        ):
            # ---- constants ----
            fa = cpool.tile([128, 2, 512], F16, name="fa")
            fb = cpool.tile([128, 2, 512], F16, name="fb")
            nc.sync.dma_start(fa[:], fa_d.ap())
            nc.sync.dma_start(fb[:], fb_d.ap())

            csm = cpool.tile([128, 2, 2, NXS], F16, name="csm")
            nc.gpsimd.dma_start(csm[:], c_d.ap())
            cr = csm[:, :, 0, :]
            ci = csm[:, :, 1, :]

            # VT replicated at partition bases 0/32/64/96 for row tiling
            vt32 = cpool.tile([128, NF], F32, name="vt32")
            for s in range(4):
                nc.sync.dma_start(vt32[32 * s : 32 * s + NBASIS, :], v_d.ap())
            vt16 = cpool.tile([128, NF], F16, name="vt16")
            nc.vector.tensor_copy(vt16[:], vt32[:])

            # resident corner-turn destinations, one per kt half
            yr = [
                cpool.tile([128, 16384], F16, name=f"yr{kt}") for kt in range(2)
            ]

            # ---- mask prefetch machinery (8192-col pair loads) ----
            mask_tiles = {}
            mask_order = [
                (kt, s, mcp) for kt in range(2) for s in range(4) for mcp in range(2)
            ]
            mask_next = [0]

            def emit_mask_loads(k, q=None):
                for _ in range(k):
                    if mask_next[0] >= len(mask_order):
                        return
                    kt, s, mcp = mask_order[mask_next[0]]
                    mask_next[0] += 1
                    c0 = kt * 65536 + s * 16384 + mcp * 2 * MCHUNK
                    m0 = wpool.tile(
                        [128, 2 * MCHUNK], U8, name=f"m0_{kt}_{s}_{mcp}", tag="m0", bufs=4
                    )
                    m1 = wpool.tile(
                        [72, 2 * MCHUNK], U8, name=f"m1_{kt}_{s}_{mcp}", tag="m1", bufs=3
                    )
                    eng = q or nc.sync
                    eng.dma_start(m0[:], m_d.ap()[0:128, c0 : c0 + 2 * MCHUNK])
                    eng.dma_start(m1[:], m_d.ap()[128:NF, c0 : c0 + 2 * MCHUNK])
                    mask_tiles[(kt, s, mcp)] = (m0, m1)

            # ---- eviction router ----
            # routes (per [*,1024] psum unit):
            #   "dd": DVE TT straight from PSUM           (1192ns DVE)
            #   "ad": Act copy -> f16 stg, DVE TT from SBUF (1038 Act + 1127 DVE)
            #   "ap": Act copy -> f16 stg, Pool TT from SBUF (1038 Act + 2127 Pool)
            # Solving for equal drain across DVE/Act/Pool in the projection
            # phase gives ~192us each at mix 71/95/90 of 256.
            rstate = {"dd": 0.0, "ad": 0.0, "ap": 0.0, "n": 0}
            rquota = {"dd": 0.28, "ad": 0.37, "ap": 0.35}

            def evict(psl, msl, obsl, pslot, nm):
                tot = rstate["n"] + 1
                rstate["n"] = tot
                route = max(rquota, key=lambda r: rquota[r] * tot - rstate[r])
                rstate[route] += 1
                if route == "dd":
                    nc.vector.tensor_tensor(obsl, psl, msl, op=MULT)
                else:
                    p = psl.shape[0]
                    stg = wpool.tile(
                        [128, 1024], F16, name=f"stg_{nm}", tag="stg", bufs=4
                    )
                    nc.scalar.copy(stg[:p, :], psl)
                    eng = nc.vector if route == "ad" else nc.gpsimd
                    eng.tensor_tensor(obsl, stg[:p, :], msl, op=MULT)

            def fft_pass_l(b):
                # x via SWDGE: Pool queue carries ONLY x loads, so desc gen
                # never head-of-line blocks, and the ~1.1us/load gen cost
                # lands on Pool's idle phase-A time.
                xb = wpool.tile([128, 2, 2, NXS], F16, name=f"xb{b}", tag="xb", bufs=3)
                nc.gpsimd.dma_start(xb[:], x_d.ap()[b])
                xr = xb[:, :, 0, :]
                xi = xb[:, :, 1, :]

                ta = wpool.tile([128, 2, NXS], F16, name=f"ta{b}", tag="ta", bufs=4)
                tb = wpool.tile([128, 2, NXS], F16, name=f"tb{b}", tag="tb", bufs=3)
                nc.vector.tensor_tensor(ta[:], xr, cr, op=MULT)
                nc.vector.tensor_tensor(tb[:], xi, ci, op=MULT)
                zr = wpool.tile([128, 2, NXS], F16, name=f"zr{b}", tag="zr", bufs=5)
                nc.vector.tensor_tensor(zr[:], ta[:], tb[:], op=SUB)
                tc_ = wpool.tile([128, 2, NXS], F16, name=f"tc{b}", tag="tc", bufs=4)
                td = wpool.tile([128, 2, NXS], F16, name=f"td{b}", tag="td", bufs=3)
                nc.vector.tensor_tensor(tc_[:], xr, ci, op=MULT)
                nc.vector.tensor_tensor(td[:], xi, cr, op=MULT)
                zi = wpool.tile([128, 2, NXS], F16, name=f"zi{b}", tag="zi", bufs=5)
                nc.vector.tensor_tensor(zi[:], tc_[:], td[:], op=ADD)

                # pass L: WT[j, k] = sum_i Z[i, j] T[i, k]   (W = T @ Z)
                # psum pl layout: [j, re k (256) | im k (256)]
                wt0 = wpool.tile(
                    [128, 2, 512], F16, name=f"wt0_{b}", tag="wt0", bufs=3
                )
                for jt in range(2):
                    js = slice(jt * 128, (jt + 1) * 128)
                    pl = psum.tile([128, 512], F32, name=f"pl{b}_{jt}", tag="ps", bufs=4)
                    nc.tensor.matmul(pl[:], zr[:, 0, js], fa[:, 0, :], start=True, stop=False)
                    nc.tensor.matmul(pl[:], zr[:, 1, js], fa[:, 1, :], start=False, stop=False)
                    nc.tensor.matmul(pl[:], zi[:, 0, js], fb[:, 0, :], start=False, stop=False)
                    nc.tensor.matmul(pl[:], zi[:, 1, js], fb[:, 1, :], start=False, stop=True)
                    nc.scalar.copy(wt0[:, jt, :], pl[:])
                return wt0

            def fft_pass_r(b, kt, wsrc):
                # pass R: Y[k, n] = sum_j WT[j, k] T[j, n]   (Y = W @ T)
                ksr = slice(kt * 128, (kt + 1) * 128)
                ksi = slice(256 + kt * 128, 256 + (kt + 1) * 128)
                pr = psum.tile([128, 512], F32, name=f"pr{b}_{kt}", tag="ps", bufs=4)
                nc.tensor.matmul(pr[:], wsrc[:, 0, ksr], fa[:, 0, :], start=True, stop=False)
                nc.tensor.matmul(pr[:], wsrc[:, 1, ksr], fa[:, 1, :], start=False, stop=False)
                nc.tensor.matmul(pr[:], wsrc[:, 0, ksi], fb[:, 0, :], start=False, stop=False)
                nc.tensor.matmul(pr[:], wsrc[:, 1, ksi], fb[:, 1, :], start=False, stop=True)
                # interleave re/im while evicting: y[.., c*2+ri] = pr[.., ri*256+c]
                yb = wpool.tile([128, 512], F16, name=f"yb{b}_{kt}", tag="yb", bufs=4)
                nc.scalar.copy(
                    yb[:].rearrange("p (c r) -> p c r", r=2),
                    pr[:].rearrange("p (r c) -> p c r", r=2),
                )
                # corner turn: yr[kt][32*(p//32) + b, (p%32)*512 + c] = yb[p, c]
                # issued from the Act queue, program-ordered right after the
                # yb copy: the dep is satisfied by construction, so the queue
                # head never stalls (SP's would, waiting on Act's copy).
                nc.scalar.dma_start(yr[kt][b : b + 97 : 32, :], yb[:])

            def project(kt, s):
                # strip s of half kt covers n in [kt*65536 + s*16384, +16384)
                n0 = kt * 65536 + s * 16384
                ysl = yr[kt][32 * s : 32 * s + NBASIS, :]
                vt0 = vt16[32 * s : 32 * s + NBASIS, 0:128]
                vt1 = vt16[32 * s : 32 * s + NBASIS, 128:NF]
                for u in range(16):  # 1024-col units across the strip
                    if u % 8 == 0:
                        m0, m1 = mask_tiles[(kt, s, u // 8)]
                        if u // 8 == 1:
                            mask_tiles.pop((kt, s, 0))
                    if u in (4, 12):
                        emit_mask_loads(1)
                    if u % 2 == 0:
                        ob0 = wpool.tile([128, 2048], F16, name=f"ob0_{kt}_{s}_{u}", tag="ob0", bufs=4)
                        ob1 = wpool.tile([72, 2048], F16, name=f"ob1_{kt}_{s}_{u}", tag="ob1", bufs=4)
                    off = u * 1024
                    osl = slice((u % 2) * 1024, (u % 2) * 1024 + 1024)
                    msl = slice((u % 8) * 1024, (u % 8) * 1024 + 1024)
                    nm = f"{kt}_{s}_{u}"
                    pp0 = psum.tile([128, 1024], F32, name=f"pp0_{nm}", tag="ps", bufs=4)
                    for h in range(2):
                        nc.tensor.matmul(
                            pp0[:, h * 512 : (h + 1) * 512],
                            vt0, ysl[:, off + h * 512 : off + (h + 1) * 512],
                            start=True, stop=True, tile_position=(32 * s, 0),
                        )
                    evict(pp0[:], m0[:, msl], ob0[:, osl], pp0, "a" + nm)
                    pp1 = psum.tile([72, 1024], F32, name=f"pp1_{nm}", tag="ps", bufs=4)
                    for h in range(2):
                        nc.tensor.matmul(
                            pp1[:, h * 512 : (h + 1) * 512],
                            vt1, ysl[:, off + h * 512 : off + (h + 1) * 512],
                            start=True, stop=True, tile_position=(32 * s, 0),
                        )
                    evict(pp1[:], m1[:, msl], ob1[:, osl], pp1, "b" + nm)
                    if u % 2 == 1:
                        c0 = n0 + (u - 1) * 1024
                        nc.sync.dma_start(o_d.ap()[0:128, c0 : c0 + 2048], ob0[:])
                        nc.sync.dma_start(o_d.ap()[128:NF, c0 : c0 + 2048], ob1[:])
                mask_tiles.pop((kt, s, 1))

            # ---- phase A: full FFT (L + both R halves), software-pipelined
            # so image b's wt0 copies hide under L(b+1)'s matmuls ----
            prev = None
            for b in range(NBASIS):
                wt0 = fft_pass_l(b)
                if prev is not None:
                    fft_pass_r(b - 1, 0, prev[:])
                    fft_pass_r(b - 1, 1, prev[:])
                prev = wt0
                if 2 <= b <= 4:
                    emit_mask_loads(1)
            fft_pass_r(NBASIS - 1, 0, prev[:])
            fft_pass_r(NBASIS - 1, 1, prev[:])
            # ---- phase B: pure projection, both kt halves ----
            for kt in range(2):
                for s in range(4):
                    project(kt, s)

    nc.compile()
    return nc


def _get_nc():
    if "nc" not in _CACHE:
        _CACHE["nc"] = _build()
    return _CACHE["nc"]


def _prep_in_maps(x, csmT, VT, maskT):
    x = np.asarray(x, dtype=np.float32)
    # [b, r, col, reim] -> [b, p, rh, reim, col]  (r = rh*128 + p)
    x = np.ascontiguousarray(
        x.reshape(NBASIS, 2, 128, NXS, 2).transpose(0, 2, 1, 4, 3).astype(np.float16)
    )
    csm = np.asarray(csmT, dtype=np.float32)
    csm = np.ascontiguousarray(
        csm.reshape(NCH, 2, 128, NXS, 2).transpose(0, 2, 1, 4, 3).astype(np.float16)
    )
    vt = np.ascontiguousarray(np.asarray(VT, dtype=np.float32))
    mask = np.ascontiguousarray(np.asarray(maskT)).view(np.uint8)
    return [{"x": x, "csm": csm[c], "vt": vt, "mask": mask} for c in range(NCH)]


def kernel(x, csmT, VT, maskT):
    from concourse import bass2jax

    nc = _get_nc()
    in_maps = _prep_in_maps(x, csmT, VT, maskT)
    results = bass2jax.run_bass_via_pjrt(nc, in_maps, n_cores=NCH)
    return np.stack(
        [results[c]["out"].astype(np.float32) for c in range(NCH)], axis=0
    )
